# revision 37
# baseline (speedup 1.0000x reference)
"""Trainium2 Bass kernel for an LSTM attention decoder (LAS-style).

Reference model per decoder step t (teacher forcing, L=128 steps):
    x_t   = emb[text[:, t-1]]            (t=0 -> emb[SOS])
    inp   = [x_t, ctx_{t-1}]             ctx_{-1} = values[:, 0, :]
    h1,c1 = LSTMCell1(inp, h1, c1)       H=1024
    h2,c2 = LSTMCell2(h1, h2, c2)        K=128
    energy= einsum('ntk,nk->nt', key, h2);  mask;  attn = softmax
    ctx_t = einsum('nt,ntv->nv', attn, values)
    pred_t= [h2, ctx_t] @ W_out.T + b_out

Sharding over 8 NeuronCores (SPMD; per-core asymmetry via inputs and the
runtime partition id):
  - LSTM1 hidden-sharded: core k owns hidden units [128k,128(k+1)) and the
    matching 512 rows of (W_ih1|W_hh1); computes its gate slice feature-major
    ([gate,batch]) for the full batch.
  - Cross-core exchange uses remote_dma_broadcast (SBUF->SBUF SDMA, no ncfw
    collective): each core broadcasts its h1.T chunk into slot `pid` of a
    shared-layout receive buffer on all 8 cores, and its 4 ctx.T columns
    into the step's slice of the gathered-ctx bank. Receivers wait on a
    monotonically increasing remote semaphore (16 per step per gather).
  - LSTM2 replicated feature-major for the full batch.
  - Attention batch-sharded via per-core key/values inputs; "my" h2 columns
    selected with a runtime-pid dynamic slice.
  - Output projection vocab-sharded and deferred: h2.T/ctx.T strips are
    banked per step; the [256, vocab/8] projection runs once at the end.
  - sigmoid(x) = 0.5*tanh(x/2)+0.5 so tanh+exp share one ACT table set.
    States are stored doubled (Hs=2h, Cs=2c) with every h-consuming weight
    pre-halved, making the sigmoid rescale free (folded into ACT scale).

Host-side numpy does memory layout only (transposes, shard slicing, dtype
casts, index shifting); all model math (matmuls, gates, softmax, embedding
gather, masking) runs on device.
"""

import contextlib
import os
import numpy as np
import ml_dtypes

_NOCRIT = bool(int(os.environ.get("KERNEL_NOCRIT", "0")))

import concourse.bacc as bacc
import concourse.bass as bass
import concourse.mybir as mybir
import concourse.tile as tile
from concourse.bass import ds, ts
from concourse.bass_utils import run_bass_kernel_spmd

F32 = mybir.dt.float32
BF16 = mybir.dt.bfloat16
I32 = mybir.dt.int32
AF = mybir.ActivationFunctionType
ALU = mybir.AluOpType

NCORES = 8
N, T, L = 32, 512, 128
V, E, H, KS, VS = 8000, 256, 1024, 128, 128
NB = N // NCORES          # 4 attention rows per core
HS = H // NCORES          # 128 hidden units per core
G4 = 4 * HS               # 512 gate rows per core (i,f,g,o chunks)
KC = 11                   # K-chunks: 0,1=x(256)  2=ctx(128)  3..10=h(1024)
VS8 = V // NCORES         # 1000 vocab rows per core
VPAD = 1024               # padded vocab shard (8*128)
NEG = -1.0e9
RD8 = [(0, d) for d in range(8)]

_CACHE = {}


def _build(n_steps=L):
    nc = bacc.Bacc(num_devices=NCORES)

    def din(name, shape, dt=F32):
        return nc.dram_tensor(name, shape, dt, kind="ExternalInput")

    w1t = din("w1t", [128, KC * G4], BF16)        # cell1 lhsT (hh part /2)
    w2t = din("w2t", [128, 9 * G4], BF16)         # cell2 lhsT (all /2)
    woutt = din("woutt", [128, 2 * VPAD], BF16)   # W_out.T shard (h2 part /2)
    emb_e = din("emb", [V, E], F32)               # full embedding table
    idx_e = din("idx", [N, L], I32)               # shifted token ids (SOS first)
    keyt = din("keyt", [128, NB * T], BF16)       # key[row].T  (k; n,c,tl)
    valt = din("valt", [128, NB * 4 * 128], BF16)  # values (tl; n,c,v)
    v0t = din("v0t", [128, N], BF16)              # values[:,0,:].T  ctx init
    b1r = din("b1r", [1, G4], F32)                # cell1 bias row (g-gate x2)
    b2r = din("b2r", [1, G4], F32)                # cell2 bias row (g-gate x2)
    borr = din("borr", [1, VPAD], F32)            # b_out shard row (padded)
    lensb = din("lensb", [128, 4 * NB], I32)      # lens broadcast (c,n)
    tgrid = din("tgrid", [128, 4 * NB], I32)      # t index grid 128c+p

    out_e = nc.dram_tensor("out", [VPAD, N * n_steps], F32, kind="ExternalOutput")

    with tile.TileContext(nc) as tc:
        with (
            tc.tile_pool(name="const", bufs=1) as cst,
            tc.tile_pool(name="work", bufs=3) as wk,
            tc.tile_pool(name="state", bufs=2) as st,
            tc.tile_pool(name="psA", bufs=2, space="PSUM") as psA,
            tc.tile_pool(name="psB", bufs=2, space="PSUM") as psB,
            tc.tile_pool(name="psS", bufs=2, space="PSUM") as psS,
        ):
            # ---------------- constants into SBUF ----------------
            c_w1t = cst.tile([128, KC * G4], BF16)
            nc.sync.dma_start(c_w1t[:], w1t[:])
            c_w1t = c_w1t.rearrange("p (k m) -> p k m", k=KC)
            c_w2t = cst.tile([128, 9 * G4], BF16)
            nc.sync.dma_start(c_w2t[:], w2t[:])
            c_w2t = c_w2t.rearrange("p (k m) -> p k m", k=9)
            c_woutt = cst.tile([128, 2 * VPAD], BF16)
            nc.sync.dma_start(c_woutt[:], woutt[:])
            c_woutt = c_woutt.rearrange("p (k m) -> p k m", k=2)
            c_keyt = cst.tile([128, NB * T], BF16)
            nc.sync.dma_start(c_keyt[:], keyt[:])
            c_keyt = c_keyt.rearrange("p (n c t) -> p n c t", n=NB, c=4)
            c_valt = cst.tile([128, NB * 4 * 128], BF16)
            nc.sync.dma_start(c_valt[:], valt[:])
            c_valt = c_valt.rearrange("p (n c v) -> p n c v", n=NB, c=4)
            c_b1r = cst.tile([1, G4], F32)
            nc.sync.dma_start(c_b1r[:], b1r[:])
            c_b2r = cst.tile([1, G4], F32)
            nc.sync.dma_start(c_b2r[:], b2r[:])
            c_borr = cst.tile([1, VPAD], F32)
            nc.sync.dma_start(c_borr[:], borr[:])

            ones_f = cst.tile([1, 512], F32)
            nc.vector.memset(ones_f[:], 1.0)
            ones_b = cst.tile([128, 1], BF16)
            nc.vector.memset(ones_b[:], 1.0)
            ones_bb = cst.tile([128, 128], BF16)
            nc.vector.memset(ones_bb[:], 1.0)
            iop = cst.tile([128, 128], I32)
            nc.gpsimd.iota(iop[:], pattern=[[0, 128]], base=0, channel_multiplier=1)
            iof = cst.tile([128, 128], I32)
            nc.gpsimd.iota(iof[:], pattern=[[1, 128]], base=0, channel_multiplier=0)
            id_f = cst.tile([128, 128], F32)
            nc.vector.tensor_tensor(id_f[:], iop[:], iof[:], op=ALU.is_equal)
            id_b = cst.tile([128, 128], BF16)
            nc.vector.tensor_copy(id_b[:], id_f[:])

            # ---------------- mask from encoder_lens ----------------
            c_lensb = cst.tile([128, 4 * NB], I32)
            nc.sync.dma_start(c_lensb[:], lensb[:])
            c_tgrid = cst.tile([128, 4 * NB], I32)
            nc.sync.dma_start(c_tgrid[:], tgrid[:])
            m01 = cst.tile([128, 4 * NB], F32)
            nc.vector.tensor_tensor(m01[:], c_tgrid[:], c_lensb[:], op=ALU.is_ge)
            maskneg = cst.tile([128, 4 * NB], BF16)
            nc.vector.tensor_scalar_mul(maskneg[:], m01[:], NEG)
            mnT_ps = psS.tile([4 * NB, 128], BF16, tag="small")
            nc.tensor.matmul(mnT_ps[:], maskneg[:], id_b[:],
                             is_transpose=True, start=True, stop=True)
            masknegT = cst.tile([4 * NB, 128], BF16)
            nc.vector.tensor_copy(masknegT[:], mnT_ps[:])

            # ---------------- embedding gather + transpose ----------------
            c_idx = cst.tile([128, N], I32)
            nc.sync.dma_start(c_idx[:], idx_e.rearrange("n l -> l n"))
            embT = cst.tile([128, 2 * N * n_steps], BF16)
            embT = embT.rearrange("p (f n t) -> p f n t", f=2, n=N)
            for n in range(N):
                eg = wk.tile([128, E], F32, tag="embg")
                nc.gpsimd.indirect_dma_start(
                    out=eg[:], out_offset=None, in_=emb_e[:],
                    in_offset=bass.IndirectOffsetOnAxis(ap=c_idx[:, n:n + 1], axis=0),
                )
                for f in range(2):
                    tp = psS.tile([128, 128], F32, tag="small")
                    nc.tensor.matmul(tp[:], eg[:, 128 * f:128 * (f + 1)], id_f[:],
                                     is_transpose=True, start=True, stop=True)
                    nc.vector.tensor_copy(embT[:, f, n, 0:n_steps], tp[:, 0:n_steps])

            # ---------------- state + banks ----------------
            c1 = cst.tile([128, N], F32)             # Cs1, feature-major shard
            nc.vector.memset(c1[:], 0.0)
            c2 = cst.tile([128, N], F32)             # Cs2, feature-major full
            nc.vector.memset(c2[:], 0.0)

            # banked Hs2.T strips; strip 0 = zeros (h2_{-1})
            s_h2t = cst.tile([128, N * (n_steps + 1)], BF16)
            nc.vector.memset(s_h2t[:, 0:N], 0.0)
            # banked gathered ctx.T strips; strip 0 = v0 (ctx_{-1})
            s_cxt = cst.tile([128, N * (n_steps + 1)], BF16)
            nc.sync.dma_start(s_cxt[:, 0:N], v0t[:])
            # gathered h1 chunks (slot = sender pid = global hidden chunk)
            h1recv = cst.tile([128, 8 * N], BF16)
            # parity-buffered broadcast sources (descriptors prepped early)
            h1src = cst.tile([128, 2 * N], BF16)
            cxsrc = cst.tile([128, 2 * NB], BF16)

            hsem = nc.alloc_semaphore("hsem")
            csem = nc.alloc_semaphore("csem")
            lsem = nc.alloc_semaphore("lsem")
            psem_h = nc.alloc_semaphore("psem_h")
            psem_c = nc.alloc_semaphore("psem_c")
            pid_p = nc.gpsimd.partition_id()
            offh = pid_p * N          # my h1 slot offset (runtime)
            offc = pid_p * NB         # my ctx slot offset (runtime)
            pid_v = nc.vector.partition_id()
            offv = pid_v * NB

            # =====================================================
            # the recurrence
            # =====================================================
            for t in range(n_steps):
                # ---- cell1: gates.T [4x128, 32], weights stationary ----
                g1 = psA.tile([128, 4 * N], F32, tag="g1")
                g1v = g1.rearrange("p (g n) -> p g n", g=4)
                # ctx chunk (kc=2) reads gathered strip t (= ctx_{t-1});
                # h chunks read h1recv slots (skip at t=0: h1_{-1}=0).
                korder = [0, 1, 2] + (list(range(3, 11)) if t > 0 else [])
                for g in range(4):
                    nc.tensor.matmul(g1v[:, g, :],
                                     c_b1r[0:1, 128 * g:128 * (g + 1)],
                                     ones_f[0:1, 0:N], start=True, stop=False)
                    for j, kc in enumerate(korder):
                        if kc < 2:
                            rhs = embT[:, kc, :, t]
                        elif kc == 2:
                            rhs = s_cxt[:, N * t:N * (t + 1)]
                        else:
                            rhs = h1recv[:, N * (kc - 3):N * (kc - 2)]
                        nc.tensor.matmul(
                            g1v[:, g, :], c_w1t[:, kc, 128 * g:128 * (g + 1)], rhs,
                            start=False, stop=(j == len(korder) - 1))

                # ---- cell1 nonlinearity (feature-major [128, 32]) ----
                t1a = wk.tile([128, 4 * N], F32, tag="t1a")
                nc.scalar.activation(t1a[:], g1[:], AF.Tanh, scale=0.5)
                ti, tf, tg, to = (t1a[:, N * g:N * (g + 1)] for g in range(4))
                q1 = wk.tile([128, N], F32, tag="q1")
                nc.vector.scalar_tensor_tensor(q1[:], tf, 1.0, c1[:],
                                               op0=ALU.add, op1=ALU.mult)
                r1 = wk.tile([128, N], F32, tag="r1")
                nc.vector.scalar_tensor_tensor(r1[:], ti, 1.0, tg,
                                               op0=ALU.add, op1=ALU.mult)
                c1n = st.tile([128, N], F32, tag="c1")
                nc.vector.scalar_tensor_tensor(c1n[:], q1[:], 0.5, r1[:],
                                               op0=ALU.mult, op1=ALU.add)
                c1 = c1n
                tc1 = wk.tile([128, N], F32, tag="tc1")
                nc.scalar.activation(tc1[:], c1[:], AF.Tanh, scale=0.5)
                h1l = h1src[:, N * (t % 2):N * (t % 2 + 1)]     # Hs1.T chunk
                nc.vector.scalar_tensor_tensor(h1l, to, 1.0, tc1[:],
                                               op0=ALU.add, op1=ALU.mult)

                # ---- gather h1 chunks via remote broadcast ----
                with (contextlib.nullcontext() if _NOCRIT
                      else tc.tile_critical()):
                    p1 = nc.gpsimd.remote_dma_broadcast(
                        h1recv[:, ds(offh, N)],
                        h1src[:, N * (t % 2):N * (t % 2 + 1)],
                        hsem, lsem, rdests=RD8)
                    np_h = t + 1
                    p1.then_inc(psem_h, 1)
                    nc.gpsimd.wait_ge(psem_h, np_h)
                    nc.gpsimd.trigger_dma(count=1)
                    nc.tensor.wait_ge(hsem, 0 if _NOCRIT else 16 * (t + 1))

                # ---- cell2: gates.T [4x128, 32], feature-major ----
                g2 = psB.tile([128, 4 * N], F32, tag="g2")
                g2v = g2.rearrange("p (g n) -> p g n", g=4)
                for g in range(4):
                    nc.tensor.matmul(g2v[:, g, :],
                                     c_b2r[0:1, 128 * g:128 * (g + 1)],
                                     ones_f[0:1, 0:N], start=True, stop=False)
                    for j in range(9):
                        rhs = (h1recv[:, N * j:N * (j + 1)] if j < 8
                               else s_h2t[:, N * t:N * (t + 1)])
                        nc.tensor.matmul(
                            g2v[:, g, :], c_w2t[:, j, 128 * g:128 * (g + 1)], rhs,
                            start=False, stop=(j == 8))

                # ---- cell2 nonlinearity (feature-major [128, 32]) ----
                t2a = wk.tile([128, 4 * N], F32, tag="t2a")
                nc.scalar.activation(t2a[:], g2[:], AF.Tanh, scale=0.5)
                u_i, u_f, u_g, u_o = (t2a[:, N * g:N * (g + 1)] for g in range(4))
                q2 = wk.tile([128, N], F32, tag="q2")
                nc.vector.scalar_tensor_tensor(q2[:], u_f, 1.0, c2[:],
                                               op0=ALU.add, op1=ALU.mult)
                r2 = wk.tile([128, N], F32, tag="r2")
                nc.vector.scalar_tensor_tensor(r2[:], u_i, 1.0, u_g,
                                               op0=ALU.add, op1=ALU.mult)
                c2n = st.tile([128, N], F32, tag="c2")
                nc.vector.scalar_tensor_tensor(c2n[:], q2[:], 0.5, r2[:],
                                               op0=ALU.mult, op1=ALU.add)
                c2 = c2n
                tc2 = wk.tile([128, N], F32, tag="tc2")
                nc.scalar.activation(tc2[:], c2[:], AF.Tanh, scale=0.5)
                # Hs2.T directly into the bank (strip t+1)
                nc.vector.scalar_tensor_tensor(
                    s_h2t[:, N * (t + 1):N * (t + 2)], u_o, 1.0, tc2[:],
                    op0=ALU.add, op1=ALU.mult)

                # my 4 columns of Hs2.T recomputed independently (dynamic
                # input slices) so attention needn't wait on the bank write
                h2my = wk.tile([128, NB], BF16, tag="h2my")
                nc.vector.scalar_tensor_tensor(
                    h2my[:], t2a[:, ds(3 * N + offv, NB)], 1.0,
                    tc2[:, ds(offv, NB)], op0=ALU.add, op1=ALU.mult)

                # ---- attention (my NB rows) ----
                en = psS.tile([128, 4 * NB], F32, tag="small")
                env = en.rearrange("p (c n) -> p c n", c=4)
                nc.tensor.matmul(en[:], masknegT[:], id_b[0:4 * NB, 0:4 * NB],
                                 start=True, stop=False)
                for nn in range(NB):
                    for cc in range(4):
                        nc.tensor.matmul(
                            env[:, cc, nn:nn + 1], c_keyt[:, nn, cc, :],
                            h2my[:, nn:nn + 1], start=False,
                            stop=(nn == NB - 1 and cc == 3))
                pe = wk.tile([128, 4 * NB], BF16, tag="pe")
                nc.scalar.activation(pe[:], en[:], AF.Exp, scale=0.5)
                pev = pe.rearrange("p (c n) -> p c n", c=4)
                sm = psS.tile([128, NB], F32, tag="small")
                for cc in range(4):
                    nc.tensor.matmul(sm[:], ones_bb[:], pev[:, cc, :],
                                     start=(cc == 0), stop=(cc == 3))
                rcbs = wk.tile([128, NB], F32, tag="rcbs")
                nc.vector.reciprocal(rcbs[:], sm[:])
                # ctx.T unnormalized [128(v), NB] — fills PE while recip runs
                cx = psS.tile([128, NB], F32, tag="small")
                for nn in range(NB):
                    for cc in range(4):
                        nc.tensor.matmul(
                            cx[:, nn:nn + 1], c_valt[:, nn, cc, :],
                            pev[:, cc, nn:nn + 1],
                            start=(cc == 0), stop=(cc == 3))
                cxT = cxsrc[:, NB * (t % 2):NB * (t % 2 + 1)]
                nc.vector.tensor_mul(cxT, cx[:], rcbs[:])

                # ---- broadcast ctx columns into gathered strip t+1 ----
                with (contextlib.nullcontext() if _NOCRIT
                      else tc.tile_critical()):
                    p2 = nc.gpsimd.remote_dma_broadcast(
                        s_cxt[:, ds(offc + N * (t + 1), NB)],
                        cxsrc[:, NB * (t % 2):NB * (t % 2 + 1)],
                        csem, lsem, rdests=RD8)
                    p2.then_inc(psem_c, 1)
                    nc.gpsimd.wait_ge(psem_c, t + 1)
                    nc.gpsimd.trigger_dma(count=1)
                    nc.tensor.wait_ge(csem, 0 if _NOCRIT else 16 * (t + 1))

                # interleaved output projection: after every 16 steps the
                # 512-column block (t-15..t) of both banks is final.
                if t % 16 == 15:
                    j = t // 16
                    for m in range(VPAD // 128):
                        pp = psB.tile([128, 512], F32, tag="g2")
                        sl = slice(512 * j, 512 * (j + 1))
                        slc = slice(N + 512 * j, N + 512 * (j + 1))
                        nc.tensor.matmul(pp[:],
                                         c_woutt[:, 0, 128 * m:128 * (m + 1)],
                                         s_h2t[:, slc], start=True, stop=False)
                        nc.tensor.matmul(pp[:],
                                         c_woutt[:, 1, 128 * m:128 * (m + 1)],
                                         s_cxt[:, slc], start=False, stop=False)
                        nc.tensor.matmul(pp[:],
                                         c_borr[:, 128 * m:128 * (m + 1)],
                                         ones_f[0:1, :], start=False, stop=True)
                        po = wk.tile([128, 512], F32, tag="po")
                        nc.vector.tensor_copy(po[:], pp[:])
                        nc.sync.dma_start(out_e[128 * m:128 * (m + 1), sl],
                                          po[:])

    nc.finalize()
    return nc


# --------------------------------------------------------------------------
# host-side sharding / layout prep (numpy only; no model math)
# --------------------------------------------------------------------------
def _prep(inputs, core, n_steps=L):
    key = np.asarray(inputs["key"], np.float32)
    values = np.asarray(inputs["values"], np.float32)
    lens = np.asarray(inputs["encoder_lens"]).astype(np.int32)
    text = np.asarray(inputs["text"]).astype(np.int32)
    emb = np.ascontiguousarray(np.asarray(inputs["emb"], np.float32))
    W_ih1 = np.asarray(inputs["W_ih1"], np.float32)
    W_hh1 = np.asarray(inputs["W_hh1"], np.float32)
    b1 = (np.asarray(inputs["b_ih1"], np.float32)
          + np.asarray(inputs["b_hh1"], np.float32))
    W_ih2 = np.asarray(inputs["W_ih2"], np.float32)
    W_hh2 = np.asarray(inputs["W_hh2"], np.float32)
    b2 = (np.asarray(inputs["b_ih2"], np.float32)
          + np.asarray(inputs["b_hh2"], np.float32))
    W_out = np.asarray(inputs["W_out"], np.float32)
    b_out = np.asarray(inputs["b_out"], np.float32)

    bf = ml_dtypes.bfloat16
    k = core
    rows4 = np.arange(NB * k, NB * (k + 1))
    hrows = np.concatenate([off + np.arange(128 * k, 128 * (k + 1))
                            for off in (0, H, 2 * H, 3 * H)])

    # cell1 lhsT: K order [x(256) | ctx(128) | h(1024)/2], shard rows.
    # gate-g rows doubled so one ACT (scale=0.5) serves all four gates.
    W1cat = np.concatenate(
        [W_ih1[:, :E], W_ih1[:, E:], 0.5 * W_hh1], axis=1)[hrows]  # [512,1408]
    W1cat = W1cat.copy()
    W1cat[256:384] *= 2.0
    w1t = np.ascontiguousarray(
        W1cat.T.reshape(KC, 128, G4).transpose(1, 0, 2).reshape(128, KC * G4))

    # cell2 lhsT: K order [h1(1024)/2 | h2(128)/2], full gate rows
    W2cat = np.concatenate([0.5 * W_ih2, 0.5 * W_hh2], axis=1)     # [512,1152]
    W2cat = W2cat.copy()
    W2cat[256:384] *= 2.0
    w2t = np.ascontiguousarray(
        W2cat.T.reshape(9, 128, G4).transpose(1, 0, 2).reshape(128, 9 * G4))

    WoT = W_out.T.copy()                                          # [256, 8000]
    WoT[:KS] *= 0.5
    sh = np.zeros((2 * 128, VPAD), np.float32)
    sh[:, :VS8] = WoT[:, VS8 * k:VS8 * (k + 1)]
    woutt = np.ascontiguousarray(
        sh.reshape(2, 128, VPAD).transpose(1, 0, 2).reshape(128, 2 * VPAD))
    borr = np.zeros((1, VPAD), np.float32)
    borr[0, :VS8] = b_out[VS8 * k:VS8 * (k + 1)]

    gd = np.repeat(np.array([1., 1., 2., 1.], np.float32), 128)
    b1r = np.ascontiguousarray((b1[hrows] * gd)[None, :])
    b2r = np.ascontiguousarray((b2 * gd)[None, :])

    keyt = np.ascontiguousarray(
        key[rows4].transpose(2, 0, 1).reshape(128, NB * T))
    valt = np.ascontiguousarray(
        values[rows4].reshape(NB, 4, 128, VS)
        .transpose(2, 0, 1, 3).reshape(128, NB * 4 * 128))
    v0t = np.ascontiguousarray(values[:, 0, :].T)                 # [128, 32]

    idx = np.empty((N, L), np.int32)
    idx[:, 0] = 1                                                 # <sos>
    idx[:, 1:] = text[:, :L - 1]
    idx = np.ascontiguousarray(idx)

    lens4 = lens[rows4]
    lensb = np.ascontiguousarray(
        np.broadcast_to(np.tile(lens4, 4)[None, :], (128, 4 * NB))).astype(np.int32)
    tgrid = np.ascontiguousarray(
        np.arange(128, dtype=np.int32)[:, None]
        + 128 * np.repeat(np.arange(4, dtype=np.int32), NB)[None, :])

    return {
        "w1t": w1t.astype(bf), "w2t": w2t.astype(bf),
        "woutt": woutt.astype(bf),
        "emb": emb, "idx": idx,
        "keyt": keyt.astype(bf), "valt": valt.astype(bf),
        "v0t": v0t.astype(bf),
        "b1r": b1r, "b2r": b2r, "borr": borr,
        "lensb": lensb, "tgrid": tgrid,
    }


def kernel(**inputs):
    n_steps = L
    if "nc" not in _CACHE:
        _CACHE["nc"] = _build(n_steps)
    nc = _CACHE["nc"]
    in_maps = [_prep(inputs, k, n_steps) for k in range(NCORES)]
    res = run_bass_kernel_spmd(nc, in_maps, core_ids=list(range(NCORES)))
    # out per core: [VPAD, (t, n)] -> its vocab shard for all (n, t)
    out = np.empty((N, n_steps, V), np.float32)
    for k in range(NCORES):
        o = res.results[k]["out"][:VS8]               # [1000, L*N]
        out[:, :, VS8 * k:VS8 * (k + 1)] = (
            o.reshape(VS8, n_steps, N).transpose(2, 1, 0))
    return out
